# revision 69
# baseline (speedup 1.0000x reference)
"""Trainium2 Bass kernel for windowed local self-attention MLP.

Reference computation (per batch b, S=2048 tokens, D=H=256, A=16, W=33):
    h   = relu(x @ W1 + b1)
    Q   = h @ Wq ; Khat = h @ Wk ; Vhat = h @ Wv        (windowed K/V are
          shifted views of Khat/Vhat -- the algebraic collapse of the
          reference's [B,S,W,H] window tensor)
    logit[s,m] = Q[s].Khat[s+A-m]/sqrt(H)  (zero outside [0,S), m=0..32)
    attn = softmax(logit) ; att[s] = sum_m attn[s,m] Vhat[s+A-m]
    out = relu(att @ Wh + bh) @ Wo + bo

Sharding: data-parallel over batch, one batch element per NeuronCore (B=8,
8 cores), weights replicated, no collectives.

Algebraic folds (host-side, float64): Wk is folded into Q via
M = Wq @ Wk^T (so K is h itself, cast to fp16), and Wh is folded into V via
Wv @ Wh (so the attention-apply directly emits pre-relu hid^T). This removes
two of the five dense matmul phases and their PSUM-drain copies.

Layout: activations feature-on-partition ([256=2x128, S]) so dense matmuls
contract over partitions; x is transposed host-side. Band attention runs per
128-token chunk over a 160-token window (= tile c + 2A rows of tile c+1 in
the shifted (-A) tiling, an exact fit for the +/-A band). The additive band
mask is pre-loaded into PSUM by an identity matmul, fp16 QK logits
accumulate on top, exp runs with a fused row-sum (accum_out), and PE
transposes of the normalized fp16 weights feed [V-tile]^T @ [e]^T fp16
matmuls whose PSUM is relu'd straight into hid. Dense matmuls use float32r
(full PE rate at N>=256, ~2e-4 rel err). Emission is a token-stripe wave
with a stage-skewed (depth-2) attention pipeline so each engine's
program-order stream never blocks on the cross-engine round trip; psum is
one shared 8-bank pool. Final projection is emitted transposed [2, S] and
streamed out per 512-token stripe; un-transposed and bo added host-side.
"""
import sys

if "/opt/trn_rl_repo" not in sys.path:
    sys.path.insert(0, "/opt/trn_rl_repo")

import numpy as np

import concourse.mybir as mybir
import concourse.tile as tile
from concourse import bacc
from concourse.bass_utils import run_bass_kernel_spmd

P = 128
S = 2048  # tokens per core
D = 256  # model dim
A = 16  # half window
NC = 16  # token chunks per core
NCORES = 8

PADW = P * (NC + 1)  # 2176: padded token axis, col = token + A
WINW = P + 2 * A  # 160: per-chunk attention window
F32 = mybir.dt.float32
F32R = mybir.dt.float32r
FP16 = mybir.dt.float16

_CACHED_NC = None
_LAST_RESULTS = None


def _build_nc():
    nc = bacc.Bacc(
        "TRN2",
        target_bir_lowering=False,
        debug=False,
        enable_asserts=False,
        num_devices=NCORES,
    )
    xt = nc.dram_tensor("xt", [D, S], F32, kind="ExternalInput").ap()
    w1 = nc.dram_tensor("w1", [D, D], F32, kind="ExternalInput").ap()
    wq = nc.dram_tensor("wq", [D, D], F32, kind="ExternalInput").ap()
    wv = nc.dram_tensor("wv", [D, D], F32, kind="ExternalInput").ap()
    wo = nc.dram_tensor("wo", [D, 2], F32, kind="ExternalInput").ap()
    b1 = nc.dram_tensor("b1", [D], F32, kind="ExternalInput").ap()
    bh = nc.dram_tensor("bh", [D], F32, kind="ExternalInput").ap()
    idh = nc.dram_tensor("idh", [P, P], FP16, kind="ExternalInput").ap()
    mkb = nc.dram_tensor("mkb", [P, 2 * WINW], FP16, kind="ExternalInput").ap()
    zer = nc.dram_tensor("zer", [P, 2 * P], F32, kind="ExternalInput").ap()
    zerh = nc.dram_tensor("zerh", [P, 2 * P], FP16, kind="ExternalInput").ap()
    out_t = nc.dram_tensor("out_t", [2, S], F32, kind="ExternalOutput").ap()

    with tile.TileContext(nc) as tc:
        with (
            tc.tile_pool(name="persist", bufs=1) as persist,
            tc.tile_pool(name="work", bufs=8) as work,
            tc.tile_pool(name="psum", bufs=8, space="PSUM") as psum,
        ):
            # ---------------- persistent tiles ----------------
            w1_sb = persist.tile([P, 2, D], F32R)
            wq_sb = persist.tile([P, 2, D], F32R)
            wv_sb = persist.tile([P, 2, D], F32R)
            wo_sb = persist.tile([P, 2, 2], F32R)
            b1_sb = persist.tile([P, 2], F32)
            bh_sb = persist.tile([P, 2], F32)
            id_h = persist.tile([P, P], FP16)
            mk_h = persist.tile([P, 2 * WINW], FP16)

            xt_sb = persist.tile([P, 2, S], F32R)
            zf16 = zerh
            ht = persist.tile([P, 2, PADW], F32R)  # col = token + A
            qt = persist.tile([P, 2, S], FP16)
            kt = persist.tile([P, 2, PADW], FP16)  # col = token + A
            vs = persist.tile([P, NC + 1, D], FP16)  # tile t row p = token t*128+p-A
            hid = persist.tile([P, 2, S], F32R)
            ot_sb = persist.tile([2, S], F32)

            def rearr(w):
                return w.rearrange("(k p) h -> p k h", p=P).bitcast(F32R)

            # ---- startup DMAs, split across both HWDGE rings ----
            xtr = xt.rearrange("(ko p) s -> p ko s", p=P).bitcast(F32R)
            nc.sync.dma_start(xt_sb[:, 0, 0:512], xtr[:, 0, 0:512])
            nc.scalar.dma_start(w1_sb[:], rearr(w1))
            nc.sync.dma_start(xt_sb[:, 1, 0:512], xtr[:, 1, 0:512])
            nc.scalar.dma_start(b1_sb[:], b1.rearrange("(hm p) -> p hm", p=P))
            nc.scalar.dma_start(wq_sb[:], rearr(wq))
            nc.scalar.dma_start(wv_sb[:], rearr(wv))
            for t in range(1, 4):
                sl = slice(t * 512, (t + 1) * 512)
                nc.sync.dma_start(xt_sb[:, 0, sl], xtr[:, 0, sl])
                nc.scalar.dma_start(xt_sb[:, 1, sl], xtr[:, 1, sl])
            # non-critical loads ride SWDGE on the otherwise-idle Pool engine
            nc.gpsimd.dma_start(id_h[:], idh)
            nc.gpsimd.dma_start(mk_h[:], mkb)
            for ko in range(2):
                nc.gpsimd.dma_start(ht[:, ko, 0:A], zer[:, 0:A].bitcast(F32R))
                nc.gpsimd.dma_start(kt[:, ko, 0:A], zf16[:, 0:A])
                nc.gpsimd.dma_start(
                    ht[:, ko, S + A:PADW], zer[:, 0:PADW - S - A].bitcast(F32R)
                )
                nc.gpsimd.dma_start(kt[:, ko, S + A:PADW], zf16[:, 0:PADW - S - A])
            nc.gpsimd.dma_start(wo_sb[:], rearr(wo))
            nc.gpsimd.dma_start(bh_sb[:], bh.rearrange("(hm p) -> p hm", p=P))

            # ---------------- per-stripe phase bodies ----------------
            def p1_stripe(t):  # ht = relu(W1^T @ xt + b1), 512 tokens
                for hm in range(2):
                    ps = psum.tile([P, 512], F32, tag="bank")
                    for k in range(2):
                        nc.tensor.matmul(
                            ps[:], w1_sb[:, k, hm * P:(hm + 1) * P],
                            xt_sb[:, k, t * 512:(t + 1) * 512],
                            start=(k == 0), stop=(k == 1),
                        )
                    if hm == 0:
                        nc.scalar.activation(
                            ht[:, hm, A + t * 512:A + (t + 1) * 512], ps[:],
                            mybir.ActivationFunctionType.Relu,
                            bias=b1_sb[:, hm:hm + 1],
                        )
                    else:
                        nc.vector.tensor_scalar(
                            ht[:, hm, A + t * 512:A + (t + 1) * 512], ps[:],
                            b1_sb[:, hm:hm + 1], 0.0,
                            mybir.AluOpType.add, mybir.AluOpType.max,
                        )

            def p23_stripe(t):  # qt = M^T ht (M = Wq Wk^T, host-folded)
                for hm in range(2):
                    psq = psum.tile([P, 512], F32, tag="bank")
                    for k in range(2):
                        nc.tensor.matmul(
                            psq[:], wq_sb[:, k, hm * P:(hm + 1) * P],
                            ht[:, k, A + t * 512:A + (t + 1) * 512],
                            start=(k == 0), stop=(k == 1),
                        )
                    nc.scalar.copy(qt[:, hm, t * 512:(t + 1) * 512], psq[:])


            def p4_pair(v0, n):  # shifted V tiles (natural layout, fp16)
                psv = psum.tile([P, 2, D], F32, tag="bank")
                for i in range(n):
                    for k in range(2):
                        nc.tensor.matmul(
                            psv[:, i, :],
                            ht[:, k, (v0 + i) * P:(v0 + i + 1) * P], wv_sb[:, k, :],
                            start=(k == 0), stop=(k == 1),
                        )
                if v0 % 4 == 2:
                    nc.scalar.copy(vs[:, v0:v0 + n, :], psv[:, 0:n, :])
                else:
                    nc.vector.tensor_copy(vs[:, v0:v0 + n, :], psv[:, 0:n, :])

            # ---- band attention, software-pipelined per chunk-pair ----
            # stage A (PE): mask-init + QK logits for both chunks into one bank
            # stage B (ACT/DVE): exp+rowsum, recip, normalize (fp16)
            # stage C (PE/DVE): transpose weights, apply V, copy att out
            pair_state = {}

            def p5_logits(cp):
                psl = psum.tile([P, 2 * WINW], F32, tag="bank", name="logit")
                nc.tensor.matmul(psl[:], id_h[:], mk_h[:], start=True, stop=False)
                for ci in range(2):
                    c = 2 * cp + ci
                    for k in range(2):
                        nc.tensor.matmul(
                            psl[:, ci * WINW:(ci + 1) * WINW],
                            qt[:, k, c * P:(c + 1) * P],
                            kt[:, k, c * P:c * P + WINW],
                            start=False, stop=(ci == 1 and k == 1),
                        )
                pair_state[cp] = psl

            def p5_softmax(cp):
                psl = pair_state.pop(cp)
                enb = work.tile([P, 2 * WINW], FP16, tag="enb")
                for ci in range(2):
                    sl = slice(ci * WINW, (ci + 1) * WINW)
                    e = work.tile([P, WINW], FP16, tag="e")
                    den = work.tile([P, 1], F32, tag="den")
                    nc.scalar.activation(
                        e[:], psl[:, sl], mybir.ActivationFunctionType.Exp,
                        scale=0.0625, accum_out=den[:],
                    )
                    rec = work.tile([P, 1], F32, tag="rec")
                    nc.vector.reciprocal(rec[:], den[:])
                    nc.vector.tensor_scalar_mul(enb[:, sl], e[:], rec[:])
                pair_state[("enb", cp)] = enb

            def p5_apply(cp):
                enb = pair_state.pop(("enb", cp))
                pse = psum.tile([P, 4, P], FP16, tag="bank", name="etr")
                for ci in range(2):
                    nc.tensor.transpose(
                        pse[:, 2 * ci, :],
                        enb[:, ci * WINW:ci * WINW + P], id_h[:]
                    )
                    nc.tensor.transpose(
                        pse[0:2 * A, 2 * ci + 1, :],
                        enb[:, ci * WINW + P:(ci + 1) * WINW], id_h[:]
                    )
                et = work.tile([P, 4, P], FP16, tag="et")
                nc.vector.tensor_copy(et[:, 0::2, :], pse[:, 0::2, :])
                nc.vector.tensor_copy(
                    et[0:2 * A, 1::2, :], pse[0:2 * A, 1::2, :]
                )
                psa = psum.tile([P, 2, 2 * P], F32, tag="bank", name="attp")
                for ci in range(2):
                    c = 2 * cp + ci
                    for fm in range(2):
                        nc.tensor.matmul(
                            psa[:, fm, ci * P:(ci + 1) * P],
                            vs[:, c, fm * P:(fm + 1) * P],
                            et[:, 2 * ci, :],
                            start=True, stop=False,
                        )
                        nc.tensor.matmul(
                            psa[:, fm, ci * P:(ci + 1) * P],
                            vs[0:2 * A, c + 1, fm * P:(fm + 1) * P],
                            et[0:2 * A, 2 * ci + 1, :],
                            start=False, stop=True,
                        )
                nc.vector.tensor_scalar(
                    hid[:, 0, cp * 2 * P:(cp + 1) * 2 * P], psa[:, 0, :],
                    bh_sb[:, 0:1], 0.0,
                    mybir.AluOpType.add, mybir.AluOpType.max,
                )
                nc.scalar.activation(
                    hid[:, 1, cp * 2 * P:(cp + 1) * 2 * P], psa[:, 1, :],
                    mybir.ActivationFunctionType.Relu, bias=bh_sb[:, 1:2],
                )

            def p7_piece(u, halves=1):  # out^T = Wo^T @ hid + stream out
                w = 512 // halves
                for hh in range(halves):
                    lo = u * 512 + hh * w
                    pso = psum.tile([2, 512], F32, tag="bank", name="outp")
                    for k in range(2):
                        nc.tensor.matmul(
                            pso[:, 0:w], wo_sb[:, k, :],
                            hid[:, k, lo:lo + w],
                            start=(k == 0), stop=(k == 1),
                        )
                    nc.scalar.copy(ot_sb[:, lo:lo + w], pso[:, 0:w])
                    nc.sync.dma_start(out_t[:, lo:lo + w], ot_sb[:, lo:lo + w])

            # ---------------- token-stripe wave + pipelined attention -------
            # Stage skew keeps each engine's stream from blocking on the
            # cross-engine round trip: logits(cp) run ~2 pair-stages ahead of
            # apply(cp).
            rounds = NC // 2
            lg = sm = ap = p6u = 0

            def flush_p6():
                nonlocal p6u
                while p6u < ap // 2:
                    p7_piece(p6u, halves=2 if p6u == 3 else 1)
                    p6u += 1

            for t in range(4):
                p1_stripe(t)
                p23_stripe(t)
                p4_pair(4 * t, 2)
                p4_pair(4 * t + 2, 2)
                if t == 3:
                    p4_pair(NC, 1)
                # K side is h itself: fp16 cast (2x_2P SBUF-to-SBUF)
                nc.vector.tensor_copy(
                    kt[:, :, A + t * 512:A + (t + 1) * 512],
                    ht[:, :, A + t * 512:A + (t + 1) * 512],
                )
                max_chunk = 4 * t + 2 if t < 3 else NC - 1
                max_lg = (max_chunk - 1) // 2
                max_ap = (4 * t + 1) // 2 if t < 3 else rounds - 1
                while lg <= max_lg:
                    p5_logits(lg)
                    lg += 1
                    if sm < lg - 1:
                        p5_softmax(sm)
                        sm += 1
                    if ap < sm - 1 and ap <= max_ap:
                        p5_apply(ap)
                        ap += 1
                        flush_p6()
            while sm < rounds:
                p5_softmax(sm)
                sm += 1
                while ap < sm - 1:
                    p5_apply(ap)
                    ap += 1
                    flush_p6()
            while ap < rounds:
                p5_apply(ap)
                ap += 1
                flush_p6()

    nc.compile()
    return nc


def _get_nc():
    global _CACHED_NC
    if _CACHED_NC is None:
        _CACHED_NC = _build_nc()
    return _CACHED_NC


def _band_mask():
    j = np.arange(WINW)[None, :]
    p = np.arange(P)[:, None]
    m = np.where((j >= p) & (j <= p + 2 * A), 0.0, -60000.0).astype(np.float16)
    return np.tile(m, (1, 2))


def kernel(x, W1, b1, Wq, Wk, Wv, Wh, bh, Wo, bo, **_unused):
    x = np.asarray(x, dtype=np.float32)
    W1 = np.asarray(W1, dtype=np.float32)
    Wq = np.asarray(Wq, dtype=np.float32)
    Wk = np.asarray(Wk, dtype=np.float32)
    Wv = np.asarray(Wv, dtype=np.float32)
    Wh = np.asarray(Wh, dtype=np.float32)
    Wo = np.asarray(Wo, dtype=np.float32)
    b1f = np.asarray(b1, dtype=np.float32).reshape(D)
    bhf = np.asarray(bh, dtype=np.float32).reshape(D)
    bof = np.asarray(bo, dtype=np.float32).reshape(2)
    zer = np.zeros((P, 2 * P), dtype=np.float32)
    zerh = np.zeros((P, 2 * P), dtype=np.float16)
    idh = np.eye(P, dtype=np.float16)
    mkb = _band_mask()

    wqm = (Wq.astype(np.float64) @ Wk.astype(np.float64).T).astype(np.float32)
    wvh = (Wv.astype(np.float64) @ Wh.astype(np.float64)).astype(np.float32)

    nc = _get_nc()
    in_maps = []
    for b in range(NCORES):
        in_maps.append({
            "xt": np.ascontiguousarray(x[b].T),
            "w1": W1, "wq": wqm, "wv": wvh, "wo": Wo,
            "b1": b1f, "bh": bhf, "zer": zer, "zerh": zerh,
            "idh": idh, "mkb": mkb,
        })
    # one retry: the shared device occasionally throws a transient
    # NRT_EXEC_UNIT_UNRECOVERABLE; re-running recovers it
    try:
        res = run_bass_kernel_spmd(nc, in_maps, core_ids=list(range(NCORES)))
    except Exception:
        res = run_bass_kernel_spmd(nc, in_maps, core_ids=list(range(NCORES)))
    global _LAST_RESULTS
    _LAST_RESULTS = res
    out = np.stack(
        [res.results[b]["out_t"].T + bof[None, :] for b in range(NCORES)], axis=0
    )
    return out.astype(np.float32)


if __name__ == "__main__":
    rng = np.random.default_rng(0)
    ins = {
        "x": rng.standard_normal((8, S, D), dtype=np.float32),
        "W1": (rng.standard_normal((D, D), dtype=np.float32) / 16),
        "b1": np.zeros((1, 1, D), np.float32),
        "Wq": (rng.standard_normal((D, D), dtype=np.float32) / 16),
        "Wk": (rng.standard_normal((D, D), dtype=np.float32) / 16),
        "Wv": (rng.standard_normal((D, D), dtype=np.float32) / 16),
        "Wh": (rng.standard_normal((D, D), dtype=np.float32) / 16),
        "bh": np.zeros((1, 1, D), np.float32),
        "Wo": (rng.standard_normal((D, 2), dtype=np.float32) / 16),
        "bo": np.zeros((1, 1, 2), np.float32),
    }
    y = kernel(**ins)
    print("kernel output", y.shape, y.dtype, float(np.abs(y).max()))
